# revision 1
# baseline (speedup 1.0000x reference)
"""Trainium2 Bass kernel for nn_CrossAttention (B=2, N=1024, M=2048, C=1024,
H=16, D=64) distributed over 8 NeuronCores.

Sharding: 2-way batch x 4-way head-group tensor parallel. Core c handles
batch b = c // 4 and heads [4*(c%4), 4*(c%4)+4). Each core computes its four
heads' normalized attention output O^T for all 1024 query rows, runs the
out-projection restricted to its own 256 Wo rows (a partial sum over the
head dimension), and a grouped ReduceScatter(add) over the 4 cores of each
batch both completes the sum over heads and hands every core its disjoint
256-query-row slice of the final output. No all-reduce, no gather.

All big matmuls run in float32r (full-rate fp32, ~1e-4 rms rounding).
Attention is computed entirely in S^T = K Q^T layout so the contraction
dimension always sits on SBUF partitions and no attention-matrix transpose
is ever materialized. Softmax skips max-subtraction (logits are LN-bounded)
and gets its denominator for free from an all-ones 65th column in the
stationary V operand. The per-(head, n) normalization happens after the
attn@V matmul on the small O^T tile via a K=1 ones-matmul broadcast.
"""

import contextlib
import sys

import numpy as np

sys.path.insert(0, "/opt/trn_rl_repo")

import concourse.mybir as mybir  # noqa: E402
import concourse.tile as tile  # noqa: E402
from concourse import bacc  # noqa: E402
from concourse.bass_utils import run_bass_kernel_spmd  # noqa: E402
from concourse.masks import make_identity  # noqa: E402

F32 = mybir.dt.float32
F32R = mybir.dt.float32r
U8 = mybir.dt.uint8
AF = mybir.ActivationFunctionType

B, N, M, C = 2, 1024, 2048, 1024
H, D = 16, 64
NHL = 4          # heads per core
NCORES = 8
EPS = 1e-6
SCALE = D ** -0.5
NLOC = 256       # output query rows per core

_CACHE = {}


def _build_program(reps=1):
    nc = bacc.Bacc("TRN2", target_bir_lowering=False, debug=False,
                   num_devices=NCORES)

    xT = nc.declare_dram_parameter("xT", [C, N], F32, isOutput=False)
    ctxT = nc.declare_dram_parameter("ctxT", [C, M], F32, isOutput=False)
    maskT = nc.declare_dram_parameter("maskT", [M, N], U8, isOutput=False)
    wq = nc.declare_dram_parameter("wq", [C, NHL * D], F32, isOutput=False)
    wk = nc.declare_dram_parameter("wk", [C, NHL * D], F32, isOutput=False)
    wv = nc.declare_dram_parameter("wv", [C, NHL * D], F32, isOutput=False)
    wo = nc.declare_dram_parameter("wo", [NHL * D, C], F32, isOutput=False)
    blkones = nc.declare_dram_parameter("blkones", [128, 2], F32, isOutput=False)
    blkq = nc.declare_dram_parameter("blkq", [2, 128], F32, isOutput=False)
    blkwk = nc.declare_dram_parameter("blkwk", [2, 2, 128], F32, isOutput=False)
    y = nc.declare_dram_parameter("y", [NLOC, C], F32, isOutput=True)

    with tile.TileContext(nc) as tc, contextlib.ExitStack() as top:
        const = top.enter_context(tc.tile_pool(name="const", bufs=1))
        persist = top.enter_context(tc.tile_pool(name="persist", bufs=1))
        dram = top.enter_context(tc.tile_pool(name="dram", bufs=1, space="DRAM"))

        # ---- constants ----
        blkones_r = const.tile([128, 2], F32R, tag="blkones")
        nc.gpsimd.dma_start(out=blkones_r[:], in_=blkones[:])
        blkq_r = const.tile([2, 128], F32R, tag="blkq")
        nc.gpsimd.dma_start(out=blkq_r[:], in_=blkq[:])
        blkwk_r = const.tile([2, 2, 128], F32R, tag="blkwk")
        nc.gpsimd.dma_start(out=blkwk_r[:], in_=blkwk[:])
        eps_t = const.tile([2, 1], F32, tag="eps")
        nc.vector.memset(eps_t[:], EPS)
        ident = const.tile([128, 128], F32, tag="ident")
        make_identity(nc, ident[:])
        ones_f = const.tile([65, 64], F32, tag="onesf")
        nc.vector.memset(ones_f[:], 1.0)
        ones_r = const.tile([65, 64], F32R, tag="onesr")
        nc.vector.tensor_copy(out=ones_r[:], in_=ones_f[:])

        # ---- persistent activations ----
        qnT = persist.tile([128, 2, N], F32R, tag="qnT")       # [2 heads x 64d, hdc, n]
        knT = persist.tile([128, 2, M], F32R, tag="knT")
        vv = persist.tile([128, NHL, 16, 65], F32R, tag="vv")  # [m-in-chunk, h, mchunk, d|1]
        maskT_sb = persist.tile([128, 16, N], U8, tag="mask")  # [m-in-chunk, mchunk, n]
        nc.sync.dma_start(out=maskT_sb[:],
                          in_=maskT[:].rearrange("(mc p) n -> p mc n", p=128))

        # ones column of the stationary V operand (softmax denominator)
        ones_col = const.tile([128, 1], F32, tag="onescol")
        nc.vector.memset(ones_col[:], 1.0)
        for h in range(NHL):
            for mc in range(16):
                nc.vector.tensor_copy(out=vv[:, h, mc, 64:65], in_=ones_col[:])

        y_part = dram.tile([N, C], F32, tag="y_part")
        y_rs = dram.tile([NLOC, C], F32, tag="y_rs")

        def _body():
            # ================= phase 1: projections + LN =================
            with contextlib.ExitStack() as s1:
                work = s1.enter_context(tc.tile_pool(name="work1", bufs=3))
                small = s1.enter_context(tc.tile_pool(name="small1", bufs=2))
                ps_proj = s1.enter_context(tc.tile_pool(name="psproj", bufs=2, space="PSUM"))
                ps_stat = s1.enter_context(tc.tile_pool(name="psstat", bufs=1, space="PSUM"))
                ps_bc = s1.enter_context(tc.tile_pool(name="psbc", bufs=1, space="PSUM"))
                ps_tr = s1.enter_context(tc.tile_pool(name="pstr", bufs=2, space="PSUM"))


                def ln_block(psum_in, out_slice, rstd_sel):
                    """LayerNorm over d=64 for a [128(=2 heads x 64d), 512] tile.

                    psum_in: PSUM [128, 512] raw projection (partition = head|d).
                    out_slice: SBUF f32r destination [128, 512].
                    rstd_sel: [2, 128] f32r selector used to broadcast rstd back
                      to 128 partitions; carries the per-(h, d) affine weight.
                    """
                    t_f = work.tile([128, 512], F32R, tag="lnt")
                    nc.scalar.copy(out=t_f[:], in_=psum_in[:])
                    sq = work.tile([128, 512], F32R, tag="lnsq")
                    nc.vector.tensor_mul(out=sq[:], in0=t_f[:], in1=t_f[:])
                    p_mean = ps_stat.tile([2, 512], F32, tag="pmean")
                    nc.tensor.matmul(p_mean[:], blkones_r[:], t_f[:], start=True, stop=True)
                    p_sq = ps_stat.tile([2, 512], F32, tag="psq")
                    nc.tensor.matmul(p_sq[:], blkones_r[:], sq[:], start=True, stop=True)
                    mu = small.tile([2, 512], F32R, tag="mu")
                    with nc.allow_low_precision(reason="LN stats in f32r"):
                        nc.scalar.mul(out=mu[:], in_=p_mean[:], mul=1.0 / 64)
                    musq = small.tile([2, 512], F32, tag="musq")
                    nc.vector.tensor_mul(out=musq[:], in0=mu[:], in1=mu[:])
                    var = small.tile([2, 512], F32, tag="var")
                    nc.scalar.mul(out=var[:], in_=p_sq[:], mul=1.0 / 64)
                    nc.vector.tensor_sub(out=var[:], in0=var[:], in1=musq[:])
                    sd = small.tile([2, 512], F32, tag="sd")
                    nc.scalar.activation(out=sd[:], in_=var[:], func=AF.Sqrt,
                                         bias=eps_t[:], scale=1.0)
                    rstd = small.tile([2, 512], F32R, tag="rstd")
                    with nc.allow_low_precision(reason="LN rstd in f32r"):
                        nc.vector.reciprocal(out=rstd[:], in_=sd[:])
                    p_mub = ps_bc.tile([128, 512], F32, tag="pmub")
                    nc.tensor.matmul(p_mub[:], blkq_r[:], mu[:], start=True, stop=True)
                    p_rstdb = ps_bc.tile([128, 512], F32, tag="prstdb")
                    nc.tensor.matmul(p_rstdb[:], rstd_sel, rstd[:], start=True, stop=True)
                    cen = work.tile([128, 512], F32, tag="lncen")
                    nc.vector.tensor_sub(out=cen[:], in0=t_f[:], in1=p_mub[:])
                    with nc.allow_low_precision(reason="normalized acts f32r"):
                        nc.vector.tensor_mul(out=out_slice, in0=cen[:], in1=p_rstdb[:])

                # Q projection + LN
                with tc.tile_pool(name="px", bufs=1) as px:
                    xT_sb = px.tile([128, 8, N], F32R, tag="xT")
                    nc.gpsimd.dma_start(out=xT_sb[:],
                                        in_=xT[:].rearrange("(cc p) n -> p cc n", p=128))
                    wq_sb = px.tile([128, 8, NHL * D], F32R, tag="wq")
                    nc.gpsimd.dma_start(out=wq_sb[:],
                                        in_=wq[:].rearrange("(cc p) h -> p cc h", p=128))
                    for hdc in range(2):
                        for nchk in range(2):
                            p_q = ps_proj.tile([128, 512], F32, tag="pproj")
                            for cc in range(8):
                                nc.tensor.matmul(
                                    p_q[:],
                                    wq_sb[:, cc, hdc * 128:(hdc + 1) * 128],
                                    xT_sb[:, cc, nchk * 512:(nchk + 1) * 512],
                                    start=(cc == 0), stop=(cc == 7))
                            ln_block(p_q, qnT[:, hdc, nchk * 512:(nchk + 1) * 512],
                                     blkq_r[:])

                pctx = s1.enter_context(tc.tile_pool(name="pctx", bufs=1))
                ctxT_sb = pctx.tile([128, 8, M], F32R, tag="ctxT")
                nc.gpsimd.dma_start(out=ctxT_sb[:],
                                    in_=ctxT[:].rearrange("(cc p) m -> p cc m", p=128))
                wk_sb = pctx.tile([128, 8, NHL * D], F32R, tag="wk")
                nc.gpsimd.dma_start(out=wk_sb[:],
                                    in_=wk[:].rearrange("(cc p) h -> p cc h", p=128))
                wv_sb = pctx.tile([128, 8, NHL * D], F32R, tag="wv")
                nc.gpsimd.dma_start(out=wv_sb[:],
                                    in_=wv[:].rearrange("(cc p) h -> p cc h", p=128))

                # K projection + LN (qn_w*kn_w product folded into rstd bcast)
                for hdc in range(2):
                    for mchk in range(4):
                        p_k = ps_proj.tile([128, 512], F32, tag="pproj")
                        for cc in range(8):
                            nc.tensor.matmul(
                                p_k[:],
                                wk_sb[:, cc, hdc * 128:(hdc + 1) * 128],
                                ctxT_sb[:, cc, mchk * 512:(mchk + 1) * 512],
                                start=(cc == 0), stop=(cc == 7))
                        ln_block(p_k, knT[:, hdc, mchk * 512:(mchk + 1) * 512],
                                 blkwk_r[:, hdc, :])

                # V projection + transpose into [m, d] stationary layout
                for hdc in range(2):
                    for mchk in range(4):
                        p_v = ps_proj.tile([128, 512], F32, tag="pproj")
                        for cc in range(8):
                            nc.tensor.matmul(
                                p_v[:],
                                wv_sb[:, cc, hdc * 128:(hdc + 1) * 128],
                                ctxT_sb[:, cc, mchk * 512:(mchk + 1) * 512],
                                start=(cc == 0), stop=(cc == 7))
                        v_f = work.tile([128, 512], F32, tag="vT")
                        nc.scalar.copy(out=v_f[:], in_=p_v[:])
                        for hp in range(2):
                            h = hdc * 2 + hp
                            lo, hi = hp * 64, hp * 64 + 64
                            for sub in range(4):
                                p_t = ps_tr.tile([128, 64], F32, tag="ptr")
                                nc.tensor.transpose(
                                    p_t[:],
                                    v_f[lo:hi, sub * 128:(sub + 1) * 128],
                                    ident[lo:hi, lo:hi])
                                nc.scalar.copy(
                                    out=vv[:, h, mchk * 4 + sub, 0:64],
                                    in_=p_t[:])

            # ================= phase 2: attention =================
            with contextlib.ExitStack() as s2o:
                late = s2o.enter_context(tc.tile_pool(name="late", bufs=1))
                # wo load overlaps attention (reuses SBUF freed by phase 1)
                oT_all = late.tile([64, NHL, N], F32R, tag="oT")   # [d, h, n]
                wo_sb = late.tile([128, 2, C], F32R, tag="wo")
                nc.gpsimd.dma_start(out=wo_sb[:],
                                    in_=wo[:].rearrange("(q p) c2 -> p q c2", p=128))

                s2 = contextlib.ExitStack()
                atp = s2.enter_context(tc.tile_pool(name="atp", bufs=3))
                rp = s2.enter_context(tc.tile_pool(name="rp", bufs=2))
                bp = s2.enter_context(tc.tile_pool(name="bp", bufs=2))
                ps_o = s2.enter_context(tc.tile_pool(name="pso", bufs=1, space="PSUM"))
                ps_s = s2.enter_context(tc.tile_pool(name="pss", bufs=2, space="PSUM"))
                ps_b = s2.enter_context(tc.tile_pool(name="psb", bufs=2, space="PSUM"))

                for nchk in range(2):
                    nsl = slice(nchk * 512, (nchk + 1) * 512)
                    p_os = [ps_o.tile([65, 512], F32, tag=f"o{h}", name=f"p_o{h}_{nchk}")
                            for h in range(NHL)]
                    for mc in range(16):
                        for h in range(NHL):
                            hdc, hp = h // 2, h % 2
                            lo, hi = hp * 64, hp * 64 + 64
                            p_s = ps_s.tile([128, 512], F32, tag="ps")
                            nc.tensor.matmul(
                                p_s[:],
                                knT[lo:hi, hdc, mc * 128:(mc + 1) * 128],
                                qnT[lo:hi, hdc, nsl],
                                start=True, stop=True)
                            at = atp.tile([128, 512], F32R, tag="at")
                            nc.scalar.activation(out=at[:], in_=p_s[:], func=AF.Exp,
                                                 scale=float(SCALE))
                            meng = nc.vector if h < 2 else nc.gpsimd
                            with nc.allow_low_precision(reason="masked probs f32r"):
                                meng.tensor_mul(out=at[:], in0=at[:],
                                                in1=maskT_sb[:, mc, nsl])
                            nc.tensor.matmul(
                                p_os[h][:], vv[:, h, mc, :], at[:],
                                start=(mc == 0), stop=(mc == 15))
                    for h in range(NHL):
                        r5 = rp.tile([65, 512], F32R, tag="r5")
                        with nc.allow_low_precision(reason="softmax recip f32r"):
                            nc.vector.reciprocal(out=r5[64:65, :],
                                                 in_=p_os[h][64:65, :])
                        p_bc = ps_b.tile([64, 512], F32, tag="pbc")
                        nc.tensor.matmul(p_bc[:], ones_r[64:65, :], r5[64:65, :],
                                         start=True, stop=True)
                        bs = bp.tile([64, 512], F32, tag="bs")
                        nc.scalar.copy(out=bs[:], in_=p_bc[:])
                        with nc.allow_low_precision(reason="attn out f32r"):
                            nc.vector.tensor_mul(out=oT_all[:, h, nsl],
                                                 in0=p_os[h][0:64, :], in1=bs[:])

                # ============ phase 3: partial out-proj + ReduceScatter ======
                s2.close()
                # Stack head pairs onto 128 partitions (DMA moves across
                # partitions; compute engines cannot).
                oT_pair = late.tile([128, 2, N], F32R, tag="oTp")
                oT_r = oT_all[:].rearrange("p (q t) n -> p q t n", t=2)
                nc.sync.dma_start(out=oT_pair[0:64, :, :], in_=oT_r[:, :, 0, :])
                nc.sync.dma_start(out=oT_pair[64:128, :, :], in_=oT_r[:, :, 1, :])

                psy = s2o.enter_context(tc.tile_pool(name="psy", bufs=4, space="PSUM"))
                yp = s2o.enter_context(tc.tile_pool(name="yp", bufs=3))
                for nn in range(8):
                    for cc2 in range(2):
                        p_y = psy.tile([128, 512], F32, tag="py")
                        for q_ in range(2):
                            nc.tensor.matmul(
                                p_y[:],
                                oT_pair[:, q_, nn * 128:(nn + 1) * 128],
                                wo_sb[:, q_, cc2 * 512:(cc2 + 1) * 512],
                                start=(q_ == 0), stop=(q_ == 1))
                        y_sb = yp.tile([128, 512], F32, tag="ysb")
                        nc.scalar.copy(out=y_sb[:], in_=p_y[:])
                        nc.sync.dma_start(
                            out=y_part[nn * 128:(nn + 1) * 128,
                                       cc2 * 512:(cc2 + 1) * 512],
                            in_=y_sb[:])

                nc.gpsimd.collective_compute(
                    "ReduceScatter", mybir.AluOpType.add,
                    replica_groups=[[0, 1, 2, 3], [4, 5, 6, 7]],
                    ins=[y_part.opt()], outs=[y_rs.opt()])
                nc.sync.dma_start(out=y[:], in_=y_rs[:])

        for _rep in range(reps):
            _body()

    nc.compile()
    return nc


def _host_prep(x, context, mask, Wq, Wkv, Wo, qn_w, kn_w):
    """Build the 8 per-core input maps."""
    x = np.asarray(x, dtype=np.float32)
    context = np.asarray(context, dtype=np.float32)
    mask_u8 = np.asarray(mask).astype(np.uint8)
    Wq = np.asarray(Wq, dtype=np.float32)
    Wkv = np.asarray(Wkv, dtype=np.float32)
    Wo = np.asarray(Wo, dtype=np.float32)
    qn_w = np.asarray(qn_w, dtype=np.float32)
    kn_w = np.asarray(kn_w, dtype=np.float32)

    Wq_r = Wq.reshape(C, H, D)
    Wkv_r = Wkv.reshape(C, 2, H, D)
    comb_w = qn_w * kn_w  # [H, D]

    blkones = np.zeros((128, 2), np.float32)
    blkones[0:64, 0] = 1.0
    blkones[64:128, 1] = 1.0
    blkq = np.zeros((2, 128), np.float32)
    blkq[0, 0:64] = 1.0
    blkq[1, 64:128] = 1.0

    in_maps = []
    for c in range(NCORES):
        b, hg = c // 4, c % 4
        heads = [4 * hg + i for i in range(NHL)]
        wq_c = np.ascontiguousarray(Wq_r[:, heads, :].reshape(C, NHL * D))
        wk_c = np.ascontiguousarray(Wkv_r[:, 0, heads, :].reshape(C, NHL * D))
        wv_c = np.ascontiguousarray(Wkv_r[:, 1, heads, :].reshape(C, NHL * D))
        # tile layout is [t(partition), hdc, col]
        blkwk = np.zeros((2, 2, 128), np.float32)
        for hdc in range(2):
            for t in range(2):
                hglob = heads[2 * hdc + t]
                blkwk[t, hdc, 64 * t:64 * t + 64] = comb_w[hglob]
        # Wo rows for local heads, in oT_pair chunk order: chunk q covers
        # local heads (2q, 2q+1); within the chunk, partitions 0-63 are head
        # 2q and 64-127 are head 2q+1.
        wo_c = np.empty((NHL * D, C), np.float32)
        for q_ in range(2):
            h0 = heads[2 * q_]
            h1 = heads[2 * q_ + 1]
            wo_c[q_ * 128:q_ * 128 + 64] = Wo[h0 * 64:(h0 + 1) * 64]
            wo_c[q_ * 128 + 64:q_ * 128 + 128] = Wo[h1 * 64:(h1 + 1) * 64]
        in_maps.append({
            "xT": np.ascontiguousarray(x[b].T),
            "ctxT": np.ascontiguousarray(context[b].T),
            "maskT": np.ascontiguousarray(mask_u8[b].T),
            "wq": wq_c, "wk": wk_c, "wv": wv_c, "wo": wo_c,
            "blkones": blkones, "blkq": blkq, "blkwk": blkwk,
        })
    return in_maps


def kernel(x, context, mask, Wq, Wkv, Wo, qn_w, kn_w):
    if "nc" not in _CACHE:
        _CACHE["nc"] = _build_program()
    nc = _CACHE["nc"]
    in_maps = _host_prep(x, context, mask, Wq, Wkv, Wo, qn_w, kn_w)
    res = run_bass_kernel_spmd(nc, in_maps, list(range(NCORES)))
    out = np.empty((B, N, C), np.float32)
    for c in range(NCORES):
        b, hg = c // 4, c % 4
        out[b, hg * NLOC:(hg + 1) * NLOC, :] = res.results[c]["y"]
    return out



# revision 4
# speedup vs baseline: 6.0953x; 6.0953x over previous
"""Trainium2 Bass kernel for nn_CrossAttention (B=2, N=1024, M=2048, C=1024,
H=16, D=64) distributed over 8 NeuronCores.

Sharding: 2-way batch x 4-way head-group tensor parallel. Core c handles
batch b = c // 4 and heads [4*(c%4), 4*(c%4)+4). Each core computes its four
heads' normalized attention output O^T for all 1024 query rows, runs the
out-projection restricted to its own 256 Wo rows (a partial sum over the
head dimension), and a grouped ReduceScatter(add) over the 4 cores of each
batch both completes the sum over heads and hands every core its disjoint
256-query-row slice of the final output.

Host->device traffic is the wall-clock bottleneck (the tunnel moves
~75 MB/s), so every input byte is shipped to exactly ONE core in fp16
(mask in u8) and the full per-core operands are reconstructed on device
with grouped AllGathers: batch groups [[0-3],[4-7]] for xT/ctxT/maskT and
pair groups [[0,4],[1,5],[2,6],[3,7]] for the per-head-group weight
slices. The program stays SPMD-uniform; per-core behavior comes entirely
from the shipped data. Output is fp16 and the ReduceScatter runs in fp16,
halving the return traffic as well.

All big matmuls run with fp16 operands (2x PE rate, fp32 PSUM accumulate).
Attention is computed entirely in S^T = K Q^T layout so the contraction
dimension always sits on SBUF partitions and no attention-matrix transpose
is ever materialized. Softmax skips max-subtraction (logits are LN-bounded,
|logit| <= 8, exp <= e^8 which also fits fp16) and gets its denominator for
free from an all-ones 65th column in the stationary V operand. V is
projected directly into [m, d] stationary layout by using ctxT chunks as
the stationary matmul operand, so no PE transposes are needed.
"""

import contextlib
import sys

import numpy as np

sys.path.insert(0, "/opt/trn_rl_repo")

import concourse.mybir as mybir  # noqa: E402
import concourse.tile as tile  # noqa: E402
from concourse import bacc  # noqa: E402
from concourse.bass_utils import run_bass_kernel_spmd  # noqa: E402

F32 = mybir.dt.float32
F32R = mybir.dt.float32r
F16 = mybir.dt.float16
U8 = mybir.dt.uint8
AF = mybir.ActivationFunctionType

B, N, M, C = 2, 1024, 2048, 1024
H, D = 16, 64
NHL = 4          # heads per core
NCORES = 8
EPS = 1e-6
SCALE = D ** -0.5
NLOC = 256       # output query rows per core

GB = [[0, 1, 2, 3], [4, 5, 6, 7]]          # batch groups (same batch)
GP = [[0, 4], [1, 5], [2, 6], [3, 7]]      # head-group pairs (same weights)

_CACHE = {}


def _build_program(reps=1):
    nc = bacc.Bacc("TRN2", target_bir_lowering=False, debug=False,
                   num_devices=NCORES)

    # Per-core shards: each is 1/len(group) of the tensor this core's group
    # reconstructs via AllGather.
    xTc = nc.declare_dram_parameter("xTc", [C * N // 4], F16, isOutput=False)
    ctxTc = nc.declare_dram_parameter("ctxTc", [C * M // 4], F16, isOutput=False)
    maskTc = nc.declare_dram_parameter("maskTc", [M * N // 4], U8, isOutput=False)
    wqc = nc.declare_dram_parameter("wqc", [C * NHL * D // 2], F16, isOutput=False)
    wkc = nc.declare_dram_parameter("wkc", [C * NHL * D // 2], F16, isOutput=False)
    wvc = nc.declare_dram_parameter("wvc", [C * NHL * D // 2], F16, isOutput=False)
    woc = nc.declare_dram_parameter("woc", [NHL * D * C // 2], F16, isOutput=False)
    blkones = nc.declare_dram_parameter("blkones", [128, 2], F32, isOutput=False)
    blkq = nc.declare_dram_parameter("blkq", [2, 128], F32, isOutput=False)
    blkwk = nc.declare_dram_parameter("blkwk", [2, 2, 128], F32, isOutput=False)
    y = nc.declare_dram_parameter("y", [NLOC, C], F16, isOutput=True)

    bypass = mybir.AluOpType.bypass

    with tile.TileContext(nc) as tc, contextlib.ExitStack() as top:
        const = top.enter_context(tc.tile_pool(name="const", bufs=1))
        persist = top.enter_context(tc.tile_pool(name="persist", bufs=1))
        dram = top.enter_context(tc.tile_pool(name="dram", bufs=1, space="DRAM"))

        # ---- gathered full operands in local DRAM ----
        xT_f = dram.tile([C, N], F16, tag="xTf")
        ctxT_f = dram.tile([C, M], F16, tag="ctxTf")
        maskT_f = dram.tile([M, N], U8, tag="maskTf")
        wq_f = dram.tile([C, NHL * D], F16, tag="wqf")
        wk_f = dram.tile([C, NHL * D], F16, tag="wkf")
        wv_f = dram.tile([C, NHL * D], F16, tag="wvf")
        wo_f = dram.tile([NHL * D, C], F16, tag="wof")

        # ---- constants ----
        blkones_r = const.tile([128, 2], F32R, tag="blkones")
        nc.gpsimd.dma_start(out=blkones_r[:], in_=blkones[:])
        blkq_r = const.tile([2, 128], F32R, tag="blkq")
        nc.gpsimd.dma_start(out=blkq_r[:], in_=blkq[:])
        blkwk_r = const.tile([2, 2, 128], F32R, tag="blkwk")
        nc.gpsimd.dma_start(out=blkwk_r[:], in_=blkwk[:])
        eps_t = const.tile([2, 1], F32, tag="eps")
        nc.vector.memset(eps_t[:], EPS)
        ones_f = const.tile([65, 64], F32, tag="onesf")
        nc.vector.memset(ones_f[:], 1.0)
        ones_r = const.tile([65, 64], F32R, tag="onesr")
        nc.vector.tensor_copy(out=ones_r[:], in_=ones_f[:])

        # ---- persistent activations ----
        qnT = persist.tile([128, 2, N], F16, tag="qnT")        # [2 heads x 64d, hdc, n]
        knT = persist.tile([128, 2, M], F16, tag="knT")
        vv = persist.tile([128, 16, NHL, 65], F16, tag="vv")   # [m-in-chunk, mchunk, h, d|1]
        maskT_sb = persist.tile([128, 16, N], U8, tag="mask")  # [m-in-chunk, mchunk, n]

        y_part = dram.tile([N, C], F16, tag="y_part")
        y_rs = dram.tile([NLOC, C], F16, tag="y_rs")

        def _gather_all():
            # Collectives can't read ExternalInput tensors; stage each chunk
            # into an Internal DRAM tile first (cheap DRAM->DRAM DMA).
            # Issued in consumption order so later ones can trail compute.
            plan = [
                ("xT", xTc, xT_f, GB),
                ("wq", wqc, wq_f, GP),
                ("ctxT", ctxTc, ctxT_f, GB),
                ("wk", wkc, wk_f, GP),
                ("wv", wvc, wv_f, GP),
                ("maskT", maskTc, maskT_f, GB),
                ("wo", woc, wo_f, GP),
            ]
            for name, src, dst, groups in plan:
                stg = dram.tile(list(src.shape), src.dtype, tag=f"stg_{name}")
                nc.sync.dma_start(out=stg, in_=src[:])
                nc.gpsimd.collective_compute(
                    "AllGather", bypass, replica_groups=groups,
                    ins=[stg.opt()], outs=[dst.opt()])

        def _body():
            nc.sync.dma_start(out=maskT_sb[:],
                              in_=maskT_f.rearrange("(mc p) n -> p mc n", p=128))
            # ================= phase 1: projections + LN =================
            with contextlib.ExitStack() as s1:
                work = s1.enter_context(tc.tile_pool(name="work1", bufs=3))
                small = s1.enter_context(tc.tile_pool(name="small1", bufs=2))
                ps_proj = s1.enter_context(tc.tile_pool(name="psproj", bufs=2, space="PSUM"))
                ps_stat = s1.enter_context(tc.tile_pool(name="psstat", bufs=1, space="PSUM"))
                ps_bc = s1.enter_context(tc.tile_pool(name="psbc", bufs=1, space="PSUM"))

                def ln_block(psum_in, out_slice, rstd_sel):
                    """LayerNorm over d=64 for a [128(=2 heads x 64d), 512] tile.

                    psum_in: PSUM [128, 512] raw projection (partition = head|d).
                    out_slice: SBUF f16 destination [128, 512].
                    rstd_sel: [2, 128] f32r selector used to broadcast rstd back
                      to 128 partitions; carries the per-(h, d) affine weight.
                    """
                    t_f = work.tile([128, 512], F32R, tag="lnt")
                    nc.scalar.copy(out=t_f[:], in_=psum_in[:])
                    sq = work.tile([128, 512], F32R, tag="lnsq")
                    nc.vector.tensor_mul(out=sq[:], in0=t_f[:], in1=t_f[:])
                    p_mean = ps_stat.tile([2, 512], F32, tag="pmean")
                    nc.tensor.matmul(p_mean[:], blkones_r[:], t_f[:], start=True, stop=True)
                    p_sq = ps_stat.tile([2, 512], F32, tag="psq")
                    nc.tensor.matmul(p_sq[:], blkones_r[:], sq[:], start=True, stop=True)
                    mu = small.tile([2, 512], F32R, tag="mu")
                    with nc.allow_low_precision(reason="LN stats in f32r"):
                        nc.scalar.mul(out=mu[:], in_=p_mean[:], mul=1.0 / 64)
                    musq = small.tile([2, 512], F32, tag="musq")
                    nc.vector.tensor_mul(out=musq[:], in0=mu[:], in1=mu[:])
                    var = small.tile([2, 512], F32, tag="var")
                    nc.scalar.mul(out=var[:], in_=p_sq[:], mul=1.0 / 64)
                    nc.vector.tensor_sub(out=var[:], in0=var[:], in1=musq[:])
                    sd = small.tile([2, 512], F32, tag="sd")
                    nc.scalar.activation(out=sd[:], in_=var[:], func=AF.Sqrt,
                                         bias=eps_t[:], scale=1.0)
                    rstd = small.tile([2, 512], F32R, tag="rstd")
                    with nc.allow_low_precision(reason="LN rstd in f32r"):
                        nc.vector.reciprocal(out=rstd[:], in_=sd[:])
                    p_mub = ps_bc.tile([128, 512], F32, tag="pmub")
                    nc.tensor.matmul(p_mub[:], blkq_r[:], mu[:], start=True, stop=True)
                    p_rstdb = ps_bc.tile([128, 512], F32, tag="prstdb")
                    nc.tensor.matmul(p_rstdb[:], rstd_sel, rstd[:], start=True, stop=True)
                    cen = work.tile([128, 512], F32, tag="lncen")
                    nc.vector.tensor_sub(out=cen[:], in0=t_f[:], in1=p_mub[:])
                    with nc.allow_low_precision(reason="normalized acts f16"):
                        nc.vector.tensor_mul(out=out_slice, in0=cen[:], in1=p_rstdb[:])

                # Q projection + LN
                with tc.tile_pool(name="px", bufs=1) as px:
                    xT_sb = px.tile([128, 8, N], F16, tag="xT")
                    nc.gpsimd.dma_start(out=xT_sb[:],
                                        in_=xT_f.rearrange("(cc p) n -> p cc n", p=128))
                    wq_sb = px.tile([128, 8, NHL * D], F16, tag="wq")
                    nc.gpsimd.dma_start(out=wq_sb[:],
                                        in_=wq_f.rearrange("(cc p) h -> p cc h", p=128))
                    for hdc in range(2):
                        for nchk in range(2):
                            p_q = ps_proj.tile([128, 512], F32, tag="pproj")
                            for cc in range(8):
                                nc.tensor.matmul(
                                    p_q[:],
                                    wq_sb[:, cc, hdc * 128:(hdc + 1) * 128],
                                    xT_sb[:, cc, nchk * 512:(nchk + 1) * 512],
                                    start=(cc == 0), stop=(cc == 7))
                            ln_block(p_q, qnT[:, hdc, nchk * 512:(nchk + 1) * 512],
                                     blkq_r[:])

                pctx = s1.enter_context(tc.tile_pool(name="pctx", bufs=1))
                ctxT_sb = pctx.tile([128, 8, M], F16, tag="ctxT")
                nc.gpsimd.dma_start(out=ctxT_sb[:],
                                    in_=ctxT_f.rearrange("(cc p) m -> p cc m", p=128))
                wk_sb = pctx.tile([128, 8, NHL * D], F16, tag="wk")
                nc.gpsimd.dma_start(out=wk_sb[:],
                                    in_=wk_f.rearrange("(cc p) h -> p cc h", p=128))
                wv_sb = pctx.tile([128, 8, NHL * D], F16, tag="wv")
                nc.gpsimd.dma_start(out=wv_sb[:],
                                    in_=wv_f.rearrange("(cc p) h -> p cc h", p=128))

                # K projection + LN (qn_w*kn_w product folded into rstd bcast)
                for hdc in range(2):
                    for mchk in range(4):
                        p_k = ps_proj.tile([128, 512], F32, tag="pproj")
                        for cc in range(8):
                            nc.tensor.matmul(
                                p_k[:],
                                wk_sb[:, cc, hdc * 128:(hdc + 1) * 128],
                                ctxT_sb[:, cc, mchk * 512:(mchk + 1) * 512],
                                start=(cc == 0), stop=(cc == 7))
                        ln_block(p_k, knT[:, hdc, mchk * 512:(mchk + 1) * 512],
                                 blkwk_r[:, hdc, :])

                # V projection straight into [m, d] stationary layout:
                # stationary = ctxT chunk [128c, 128m], moving = wv [128c, 256hd]
                # -> PSUM [128m, 256hd]; col 64 of each head slot in vv stays
                # the memset 1.0 (softmax denominator).
                nc.vector.memset(vv[:], 1.0)
                for mc in range(16):
                    p_v = ps_proj.tile([128, 256], F32, tag="pprojv")
                    for cc in range(8):
                        nc.tensor.matmul(
                            p_v[:],
                            ctxT_sb[:, cc, mc * 128:(mc + 1) * 128],
                            wv_sb[:, cc, :],
                            start=(cc == 0), stop=(cc == 7))
                    with nc.allow_low_precision(reason="V in f16"):
                        nc.scalar.copy(
                            out=vv[:, mc, :, 0:64],
                            in_=p_v[:].rearrange("p (h d) -> p h d", h=NHL))

            # ================= phase 2: attention =================
            with contextlib.ExitStack() as s2o:
                late = s2o.enter_context(tc.tile_pool(name="late", bufs=1))
                # wo load overlaps attention (reuses SBUF freed by phase 1)
                oT_all = late.tile([64, NHL, N], F16, tag="oT")   # [d, h, n]
                wo_sb = late.tile([128, 2, C], F16, tag="wo")
                nc.gpsimd.dma_start(out=wo_sb[:],
                                    in_=wo_f.rearrange("(q p) c2 -> p q c2", p=128))

                s2 = contextlib.ExitStack()
                atp = s2.enter_context(tc.tile_pool(name="atp", bufs=3))
                rp = s2.enter_context(tc.tile_pool(name="rp", bufs=2))
                bp = s2.enter_context(tc.tile_pool(name="bp", bufs=2))
                ps_o = s2.enter_context(tc.tile_pool(name="pso", bufs=1, space="PSUM"))
                ps_s = s2.enter_context(tc.tile_pool(name="pss", bufs=2, space="PSUM"))
                ps_b = s2.enter_context(tc.tile_pool(name="psb", bufs=2, space="PSUM"))

                for nchk in range(2):
                    nsl = slice(nchk * 512, (nchk + 1) * 512)
                    p_os = [ps_o.tile([65, 512], F32, tag=f"o{h}", name=f"p_o{h}_{nchk}")
                            for h in range(NHL)]
                    for mc in range(16):
                        for h in range(NHL):
                            hdc, hp = h // 2, h % 2
                            lo, hi = hp * 64, hp * 64 + 64
                            p_s = ps_s.tile([128, 512], F32, tag="ps")
                            nc.tensor.matmul(
                                p_s[:],
                                knT[lo:hi, hdc, mc * 128:(mc + 1) * 128],
                                qnT[lo:hi, hdc, nsl],
                                start=True, stop=True)
                            at = atp.tile([128, 512], F16, tag="at")
                            nc.scalar.activation(out=at[:], in_=p_s[:], func=AF.Exp,
                                                 scale=float(SCALE))
                            meng = nc.vector if h < 2 else nc.gpsimd
                            with nc.allow_low_precision(reason="masked probs f16"):
                                meng.tensor_mul(out=at[:], in0=at[:],
                                                in1=maskT_sb[:, mc, nsl])
                            nc.tensor.matmul(
                                p_os[h][:], vv[:, mc, h, :], at[:],
                                start=(mc == 0), stop=(mc == 15))
                    for h in range(NHL):
                        r5 = rp.tile([65, 512], F32R, tag="r5")
                        with nc.allow_low_precision(reason="softmax recip f32r"):
                            nc.vector.reciprocal(out=r5[64:65, :],
                                                 in_=p_os[h][64:65, :])
                        p_bc = ps_b.tile([64, 512], F32, tag="pbc")
                        nc.tensor.matmul(p_bc[:], ones_r[64:65, :], r5[64:65, :],
                                         start=True, stop=True)
                        bs = bp.tile([64, 512], F32, tag="bs")
                        nc.scalar.copy(out=bs[:], in_=p_bc[:])
                        with nc.allow_low_precision(reason="attn out f16"):
                            nc.vector.tensor_mul(out=oT_all[:, h, nsl],
                                                 in0=p_os[h][0:64, :], in1=bs[:])

                # ============ phase 3: partial out-proj + ReduceScatter ======
                s2.close()
                # Stack head pairs onto 128 partitions (DMA moves across
                # partitions; compute engines cannot).
                oT_pair = late.tile([128, 2, N], F16, tag="oTp")
                oT_r = oT_all[:].rearrange("p (q t) n -> p q t n", t=2)
                nc.sync.dma_start(out=oT_pair[0:64, :, :], in_=oT_r[:, :, 0, :])
                nc.sync.dma_start(out=oT_pair[64:128, :, :], in_=oT_r[:, :, 1, :])

                psy = s2o.enter_context(tc.tile_pool(name="psy", bufs=4, space="PSUM"))
                yp = s2o.enter_context(tc.tile_pool(name="yp", bufs=3))
                for nn in range(8):
                    for cc2 in range(2):
                        p_y = psy.tile([128, 512], F32, tag="py")
                        for q_ in range(2):
                            nc.tensor.matmul(
                                p_y[:],
                                oT_pair[:, q_, nn * 128:(nn + 1) * 128],
                                wo_sb[:, q_, cc2 * 512:(cc2 + 1) * 512],
                                start=(q_ == 0), stop=(q_ == 1))
                        y_sb = yp.tile([128, 512], F16, tag="ysb")
                        with nc.allow_low_precision(reason="partial y f16"):
                            nc.scalar.copy(out=y_sb[:], in_=p_y[:])
                        nc.sync.dma_start(
                            out=y_part[nn * 128:(nn + 1) * 128,
                                       cc2 * 512:(cc2 + 1) * 512],
                            in_=y_sb[:])

                nc.gpsimd.collective_compute(
                    "ReduceScatter", mybir.AluOpType.add,
                    replica_groups=GB,
                    ins=[y_part.opt()], outs=[y_rs.opt()])
                nc.sync.dma_start(out=y[:], in_=y_rs[:])

        _gather_all()
        for _rep in range(reps):
            _body()

    nc.compile()
    return nc


def _host_prep(x, context, mask, Wq, Wkv, Wo, qn_w, kn_w):
    """Build the 8 per-core input maps (fp16 shards, one byte -> one core)."""
    x = np.asarray(x)
    context = np.asarray(context)
    mask = np.asarray(mask)
    Wq = np.asarray(Wq, dtype=np.float32)
    Wkv = np.asarray(Wkv, dtype=np.float32)
    Wo = np.asarray(Wo, dtype=np.float32)
    qn_w = np.asarray(qn_w, dtype=np.float32)
    kn_w = np.asarray(kn_w, dtype=np.float32)

    # [C, N]/[C, M]/[M, N] fp16/u8, flattened per batch
    xT16 = [x[b].T.astype(np.float16).reshape(-1) for b in range(B)]
    ctxT16 = [context[b].T.astype(np.float16).reshape(-1) for b in range(B)]
    maskT8 = [mask[b].T.astype(np.uint8).reshape(-1) for b in range(B)]

    Wq_r = Wq.reshape(C, H, D)
    Wkv_r = Wkv.reshape(C, 2, H, D)
    comb_w = qn_w * kn_w  # [H, D]

    # per-head-group fp16 weight slices, flattened
    wq_hg, wk_hg, wv_hg, wo_hg = [], [], [], []
    for hg in range(4):
        sl = slice(4 * hg, 4 * hg + 4)
        wq_hg.append(Wq_r[:, sl, :].astype(np.float16).reshape(-1))
        wk_hg.append(Wkv_r[:, 0, sl, :].astype(np.float16).reshape(-1))
        wv_hg.append(Wkv_r[:, 1, sl, :].astype(np.float16).reshape(-1))
        wo_hg.append(Wo[4 * hg * D:(4 * hg + 4) * D].astype(np.float16).reshape(-1))

    blkones = np.zeros((128, 2), np.float32)
    blkones[0:64, 0] = 1.0
    blkones[64:128, 1] = 1.0
    blkq = np.zeros((2, 128), np.float32)
    blkq[0, 0:64] = 1.0
    blkq[1, 64:128] = 1.0

    XCH = C * N // 4
    CCH = C * M // 4
    MCH = M * N // 4
    WCH = C * NHL * D // 2

    in_maps = []
    for c in range(NCORES):
        b, hg = c // 4, c % 4
        r = c % 4       # rank within batch group
        r2 = c // 4     # rank within head-group pair
        heads = [4 * hg + i for i in range(NHL)]
        # tile layout is [t(partition), hdc, col]
        blkwk = np.zeros((2, 2, 128), np.float32)
        for hdc in range(2):
            for t in range(2):
                hglob = heads[2 * hdc + t]
                blkwk[t, hdc, 64 * t:64 * t + 64] = comb_w[hglob]
        in_maps.append({
            "xTc": xT16[b][r * XCH:(r + 1) * XCH],
            "ctxTc": ctxT16[b][r * CCH:(r + 1) * CCH],
            "maskTc": maskT8[b][r * MCH:(r + 1) * MCH],
            "wqc": wq_hg[hg][r2 * WCH:(r2 + 1) * WCH],
            "wkc": wk_hg[hg][r2 * WCH:(r2 + 1) * WCH],
            "wvc": wv_hg[hg][r2 * WCH:(r2 + 1) * WCH],
            "woc": wo_hg[hg][r2 * WCH:(r2 + 1) * WCH],
            "blkones": blkones, "blkq": blkq, "blkwk": blkwk,
        })
    return in_maps


def kernel(x, context, mask, Wq, Wkv, Wo, qn_w, kn_w):
    if "nc" not in _CACHE:
        _CACHE["nc"] = _build_program()
    nc = _CACHE["nc"]
    in_maps = _host_prep(x, context, mask, Wq, Wkv, Wo, qn_w, kn_w)
    res = run_bass_kernel_spmd(nc, in_maps, list(range(NCORES)))
    out = np.empty((B, N, C), np.float32)
    for c in range(NCORES):
        b, hg = c // 4, c % 4
        out[b, hg * NLOC:(hg + 1) * NLOC, :] = res.results[c]["y"].astype(np.float32)
    return out


# revision 11
# speedup vs baseline: 6.1541x; 1.0096x over previous
"""Trainium2 Bass kernel for nn_CrossAttention (B=2, N=1024, M=2048, C=1024,
H=16, D=64) distributed over 8 NeuronCores.

Sharding: 2-way batch x 4-way head-group tensor parallel. Core c handles
batch b = c // 4 and heads [4*(c%4), 4*(c%4)+4). Each core computes its four
heads' normalized attention output O^T for all 1024 query rows, runs the
out-projection restricted to its own 256 Wo rows (a partial sum over the
head dimension), and a grouped ReduceScatter(add) over the 4 cores of each
batch both completes the sum over heads and hands every core its disjoint
256-query-row slice of the final output.

Host->device traffic is the wall-clock bottleneck (the tunnel moves
~75 MB/s), so every input byte is shipped to exactly ONE core in fp16
(mask in u8) and the full per-core operands are reconstructed on device
with grouped AllGathers: batch groups [[0-3],[4-7]] for xT/ctxT/maskT and
pair groups [[0,4],[1,5],[2,6],[3,7]] for the per-head-group weight
slices. The program stays SPMD-uniform; per-core behavior comes entirely
from the shipped data. Output is fp16 and the ReduceScatter runs in fp16,
halving the return traffic as well.

All big matmuls run with fp16 operands (2x PE rate, fp32 PSUM accumulate).
Attention is computed entirely in S^T = K Q^T layout so the contraction
dimension always sits on SBUF partitions and no attention-matrix transpose
is ever materialized. Softmax skips max-subtraction (logits are LN-bounded,
|logit| <= 8, exp <= e^8 which also fits fp16) and gets its denominator for
free from an all-ones 65th column in the stationary V operand. V is
projected directly into [m, d] stationary layout by using ctxT chunks as
the stationary matmul operand, so no PE transposes are needed.
"""

import contextlib
import sys

import numpy as np

sys.path.insert(0, "/opt/trn_rl_repo")

import concourse.mybir as mybir  # noqa: E402
import concourse.tile as tile  # noqa: E402
from concourse import bacc  # noqa: E402
from concourse.bass_utils import run_bass_kernel_spmd  # noqa: E402

F32 = mybir.dt.float32
F32R = mybir.dt.float32r
F16 = mybir.dt.float16
U8 = mybir.dt.uint8
AF = mybir.ActivationFunctionType

B, N, M, C = 2, 1024, 2048, 1024
H, D = 16, 64
NHL = 4          # heads per core
NCORES = 8
EPS = 1e-6
SCALE = D ** -0.5
NLOC = 256       # output query rows per core

GB = [[0, 1, 2, 3], [4, 5, 6, 7]]          # batch groups (same batch)
GP = [[0, 4], [1, 5], [2, 6], [3, 7]]      # head-group pairs (same weights)

_CACHE = {}


def _build_program(reps=1):
    nc = bacc.Bacc("TRN2", target_bir_lowering=False, debug=False,
                   num_devices=NCORES)

    # Per-core shards: each is 1/len(group) of the tensor this core's group
    # reconstructs via AllGather.
    xTc = nc.declare_dram_parameter("xTc", [C * N // 4], F16, isOutput=False)
    ctxTc = nc.declare_dram_parameter("ctxTc", [C * M // 4], F16, isOutput=False)
    # mask bitpacked along n (little bitorder): [M, N/8] u8, 1/4 per core
    maskTc = nc.declare_dram_parameter("maskTc", [M * N // 8 // 4], U8,
                                       isOutput=False)
    wqc = nc.declare_dram_parameter("wqc", [C * NHL * D // 2], F16, isOutput=False)
    wkc = nc.declare_dram_parameter("wkc", [C * NHL * D // 2], F16, isOutput=False)
    wvc = nc.declare_dram_parameter("wvc", [C * NHL * D // 2], F16, isOutput=False)
    woc = nc.declare_dram_parameter("woc", [NHL * D * C // 2], F16, isOutput=False)
    blkones = nc.declare_dram_parameter("blkones", [128, 2], F32, isOutput=False)
    blkq = nc.declare_dram_parameter("blkq", [2, 128], F32, isOutput=False)
    blkwk = nc.declare_dram_parameter("blkwk", [2, 2, 128], F32, isOutput=False)
    y = nc.declare_dram_parameter("y", [NLOC, C], F16, isOutput=True)

    bypass = mybir.AluOpType.bypass

    with tile.TileContext(nc) as tc, contextlib.ExitStack() as top:
        const = top.enter_context(tc.tile_pool(name="const", bufs=1))
        persist = top.enter_context(tc.tile_pool(name="persist", bufs=1))
        dram = top.enter_context(tc.tile_pool(name="dram", bufs=1, space="DRAM"))

        # ---- gathered full operands in local DRAM ----
        xT_f = dram.tile([C, N], F16, tag="xTf")
        ctxT_f = dram.tile([C, M], F16, tag="ctxTf")
        maskP_f = dram.tile([M, N // 8], U8, tag="maskPf")
        wq_f = dram.tile([C, NHL * D], F16, tag="wqf")
        wk_f = dram.tile([C, NHL * D], F16, tag="wkf")
        wv_f = dram.tile([C, NHL * D], F16, tag="wvf")
        wo_f = dram.tile([NHL * D, C], F16, tag="wof")

        # ---- constants ----
        blkones_r = const.tile([128, 2], F32R, tag="blkones")
        nc.gpsimd.dma_start(out=blkones_r[:], in_=blkones[:])
        blkq_r = const.tile([2, 128], F32R, tag="blkq")
        nc.gpsimd.dma_start(out=blkq_r[:], in_=blkq[:])
        blkwk_r = const.tile([2, 2, 128], F32R, tag="blkwk")
        nc.gpsimd.dma_start(out=blkwk_r[:], in_=blkwk[:])
        eps_t = const.tile([2, 1], F32, tag="eps")
        nc.vector.memset(eps_t[:], EPS)
        ones_f = const.tile([65, 64], F32, tag="onesf")
        nc.vector.memset(ones_f[:], 1.0)
        ones_r = const.tile([65, 64], F32R, tag="onesr")
        nc.vector.tensor_copy(out=ones_r[:], in_=ones_f[:])

        # ---- persistent activations ----
        qnT = persist.tile([128, 2, N], F16, tag="qnT")        # [2 heads x 64d, hdc, n]
        knT = persist.tile([128, 2, M], F16, tag="knT")
        vv = persist.tile([128, 16, NHL, 65], F16, tag="vv")   # [m-in-chunk, mchunk, h, d|1]
        maskT_sb = persist.tile([128, 16, N], U8, tag="mask")  # [m-in-chunk, mchunk, n]

        y_part = dram.tile([N, C], F16, tag="y_part")
        y_rs = dram.tile([NLOC, C], F16, tag="y_rs")

        def _gather_all():
            # Collectives can't read ExternalInput tensors; stage each chunk
            # into an Internal DRAM tile first (cheap DRAM->DRAM DMA).
            # Issued in consumption order so later ones can trail compute.
            plan = [
                ("xT", xTc, xT_f, GB),
                ("wq", wqc, wq_f, GP),
                ("ctxT", ctxTc, ctxT_f, GB),
                ("wk", wkc, wk_f, GP),
                ("wv", wvc, wv_f, GP),
                ("maskP", maskTc, maskP_f, GB),
                ("wo", woc, wo_f, GP),
            ]
            for name, src, dst, groups in plan:
                stg = dram.tile(list(src.shape), src.dtype, tag=f"stg_{name}")
                nc.sync.dma_start(out=stg, in_=src[:])
                nc.gpsimd.collective_compute(
                    "AllGather", bypass, replica_groups=groups,
                    ins=[stg.opt()], outs=[dst.opt()])

        def _body():
            # Load packed mask bits and unpack to u8 {0,1}: n = 8*i + j
            # (little bitorder), so slice j of the (i j) split gets
            # (byte >> j) & 1 in one two-op tensor_scalar each.
            maskP_sb = persist.tile([128, 16, N // 8], U8, tag="maskP")
            nc.sync.dma_start(out=maskP_sb[:],
                              in_=maskP_f.rearrange("(mc p) nb -> p mc nb", p=128))
            mview = maskT_sb[:].rearrange("p mc (i j) -> p mc i j", j=8)
            for j in range(8):
                nc.vector.tensor_scalar(
                    out=mview[:, :, :, j], in0=maskP_sb[:],
                    scalar1=j, scalar2=1,
                    op0=mybir.AluOpType.logical_shift_right,
                    op1=mybir.AluOpType.bitwise_and)
            # ================= phase 1: projections + LN =================
            with contextlib.ExitStack() as s1:
                work = s1.enter_context(tc.tile_pool(name="work1", bufs=3))
                small = s1.enter_context(tc.tile_pool(name="small1", bufs=2))
                ps_proj = s1.enter_context(tc.tile_pool(name="psproj", bufs=2, space="PSUM"))
                ps_stat = s1.enter_context(tc.tile_pool(name="psstat", bufs=1, space="PSUM"))
                ps_bc = s1.enter_context(tc.tile_pool(name="psbc", bufs=1, space="PSUM"))

                def ln_block(psum_in, out_slice, rstd_sel):
                    """LayerNorm over d=64 for a [128(=2 heads x 64d), 512] tile.

                    psum_in: PSUM [128, 512] raw projection (partition = head|d).
                    out_slice: SBUF f16 destination [128, 512].
                    rstd_sel: [2, 128] f32r selector used to broadcast rstd back
                      to 128 partitions; carries the per-(h, d) affine weight.
                    """
                    t_f = work.tile([128, 512], F32R, tag="lnt")
                    nc.scalar.copy(out=t_f[:], in_=psum_in[:])
                    sq = work.tile([128, 512], F32R, tag="lnsq")
                    nc.vector.tensor_mul(out=sq[:], in0=t_f[:], in1=t_f[:])
                    p_mean = ps_stat.tile([2, 512], F32, tag="pmean")
                    nc.tensor.matmul(p_mean[:], blkones_r[:], t_f[:], start=True, stop=True)
                    p_sq = ps_stat.tile([2, 512], F32, tag="psq")
                    nc.tensor.matmul(p_sq[:], blkones_r[:], sq[:], start=True, stop=True)
                    mu = small.tile([2, 512], F32R, tag="mu")
                    with nc.allow_low_precision(reason="LN stats in f32r"):
                        nc.scalar.mul(out=mu[:], in_=p_mean[:], mul=1.0 / 64)
                    musq = small.tile([2, 512], F32, tag="musq")
                    nc.vector.tensor_mul(out=musq[:], in0=mu[:], in1=mu[:])
                    var = small.tile([2, 512], F32, tag="var")
                    nc.scalar.mul(out=var[:], in_=p_sq[:], mul=1.0 / 64)
                    nc.vector.tensor_sub(out=var[:], in0=var[:], in1=musq[:])
                    sd = small.tile([2, 512], F32, tag="sd")
                    nc.scalar.activation(out=sd[:], in_=var[:], func=AF.Sqrt,
                                         bias=eps_t[:], scale=1.0)
                    rstd = small.tile([2, 512], F32R, tag="rstd")
                    with nc.allow_low_precision(reason="LN rstd in f32r"):
                        nc.vector.reciprocal(out=rstd[:], in_=sd[:])
                    p_mub = ps_bc.tile([128, 512], F32, tag="pmub")
                    nc.tensor.matmul(p_mub[:], blkq_r[:], mu[:], start=True, stop=True)
                    p_rstdb = ps_bc.tile([128, 512], F32, tag="prstdb")
                    nc.tensor.matmul(p_rstdb[:], rstd_sel, rstd[:], start=True, stop=True)
                    cen = work.tile([128, 512], F32, tag="lncen")
                    nc.vector.tensor_sub(out=cen[:], in0=t_f[:], in1=p_mub[:])
                    with nc.allow_low_precision(reason="normalized acts f16"):
                        nc.vector.tensor_mul(out=out_slice, in0=cen[:], in1=p_rstdb[:])

                # Q projection + LN
                with tc.tile_pool(name="px", bufs=1) as px:
                    xT_sb = px.tile([128, 8, N], F16, tag="xT")
                    nc.gpsimd.dma_start(out=xT_sb[:],
                                        in_=xT_f.rearrange("(cc p) n -> p cc n", p=128))
                    wq_sb = px.tile([128, 8, NHL * D], F16, tag="wq")
                    nc.gpsimd.dma_start(out=wq_sb[:],
                                        in_=wq_f.rearrange("(cc p) h -> p cc h", p=128))
                    for hdc in range(2):
                        for nchk in range(2):
                            p_q = ps_proj.tile([128, 512], F32, tag="pproj")
                            for cc in range(8):
                                nc.tensor.matmul(
                                    p_q[:],
                                    wq_sb[:, cc, hdc * 128:(hdc + 1) * 128],
                                    xT_sb[:, cc, nchk * 512:(nchk + 1) * 512],
                                    start=(cc == 0), stop=(cc == 7))
                            ln_block(p_q, qnT[:, hdc, nchk * 512:(nchk + 1) * 512],
                                     blkq_r[:])

                pctx = s1.enter_context(tc.tile_pool(name="pctx", bufs=1))
                ctxT_sb = pctx.tile([128, 8, M], F16, tag="ctxT")
                nc.gpsimd.dma_start(out=ctxT_sb[:],
                                    in_=ctxT_f.rearrange("(cc p) m -> p cc m", p=128))
                wk_sb = pctx.tile([128, 8, NHL * D], F16, tag="wk")
                nc.gpsimd.dma_start(out=wk_sb[:],
                                    in_=wk_f.rearrange("(cc p) h -> p cc h", p=128))
                wv_sb = pctx.tile([128, 8, NHL * D], F16, tag="wv")
                nc.gpsimd.dma_start(out=wv_sb[:],
                                    in_=wv_f.rearrange("(cc p) h -> p cc h", p=128))

                # K projection + LN (qn_w*kn_w product folded into rstd bcast)
                for hdc in range(2):
                    for mchk in range(4):
                        p_k = ps_proj.tile([128, 512], F32, tag="pproj")
                        for cc in range(8):
                            nc.tensor.matmul(
                                p_k[:],
                                wk_sb[:, cc, hdc * 128:(hdc + 1) * 128],
                                ctxT_sb[:, cc, mchk * 512:(mchk + 1) * 512],
                                start=(cc == 0), stop=(cc == 7))
                        ln_block(p_k, knT[:, hdc, mchk * 512:(mchk + 1) * 512],
                                 blkwk_r[:, hdc, :])

                # V projection straight into [m, d] stationary layout:
                # stationary = ctxT chunk [128c, 128m], moving = wv [128c, 256hd]
                # -> PSUM [128m, 256hd]; col 64 of each head slot in vv stays
                # the memset 1.0 (softmax denominator).
                nc.vector.memset(vv[:], 1.0)
                for mc in range(16):
                    p_v = ps_proj.tile([128, 256], F32, tag="pprojv")
                    for cc in range(8):
                        nc.tensor.matmul(
                            p_v[:],
                            ctxT_sb[:, cc, mc * 128:(mc + 1) * 128],
                            wv_sb[:, cc, :],
                            start=(cc == 0), stop=(cc == 7))
                    with nc.allow_low_precision(reason="V in f16"):
                        nc.scalar.copy(
                            out=vv[:, mc, :, 0:64],
                            in_=p_v[:].rearrange("p (h d) -> p h d", h=NHL))

            # ================= phase 2: attention =================
            with contextlib.ExitStack() as s2o:
                late = s2o.enter_context(tc.tile_pool(name="late", bufs=1))
                # wo load overlaps attention (reuses SBUF freed by phase 1)
                oT_all = late.tile([64, NHL, N], F16, tag="oT")   # [d, h, n]
                wo_sb = late.tile([128, 2, C], F16, tag="wo")
                nc.gpsimd.dma_start(out=wo_sb[:],
                                    in_=wo_f.rearrange("(q p) c2 -> p q c2", p=128))

                s2 = contextlib.ExitStack()
                atp = s2.enter_context(tc.tile_pool(name="atp", bufs=3))
                rp = s2.enter_context(tc.tile_pool(name="rp", bufs=2))
                bp = s2.enter_context(tc.tile_pool(name="bp", bufs=2))
                ps_o = s2.enter_context(tc.tile_pool(name="pso", bufs=1, space="PSUM"))
                ps_s = s2.enter_context(tc.tile_pool(name="pss", bufs=2, space="PSUM"))
                ps_b = s2.enter_context(tc.tile_pool(name="psb", bufs=2, space="PSUM"))

                for nchk in range(2):
                    nsl = slice(nchk * 512, (nchk + 1) * 512)
                    p_os = [ps_o.tile([65, 512], F32, tag=f"o{h}", name=f"p_o{h}_{nchk}")
                            for h in range(NHL)]
                    for mc in range(16):
                        for h in range(NHL):
                            hdc, hp = h // 2, h % 2
                            lo, hi = hp * 64, hp * 64 + 64
                            p_s = ps_s.tile([128, 512], F32, tag="ps")
                            nc.tensor.matmul(
                                p_s[:],
                                knT[lo:hi, hdc, mc * 128:(mc + 1) * 128],
                                qnT[lo:hi, hdc, nsl],
                                start=True, stop=True)
                            at = atp.tile([128, 512], F16, tag="at")
                            nc.scalar.activation(out=at[:], in_=p_s[:], func=AF.Exp,
                                                 scale=float(SCALE))
                            meng = nc.vector if h < 2 else nc.gpsimd
                            with nc.allow_low_precision(reason="masked probs f16"):
                                meng.tensor_mul(out=at[:], in0=at[:],
                                                in1=maskT_sb[:, mc, nsl])
                            nc.tensor.matmul(
                                p_os[h][:], vv[:, mc, h, :], at[:],
                                start=(mc == 0), stop=(mc == 15))
                    for h in range(NHL):
                        r5 = rp.tile([65, 512], F32R, tag="r5")
                        with nc.allow_low_precision(reason="softmax recip f32r"):
                            nc.vector.reciprocal(out=r5[64:65, :],
                                                 in_=p_os[h][64:65, :])
                        p_bc = ps_b.tile([64, 512], F32, tag="pbc")
                        nc.tensor.matmul(p_bc[:], ones_r[64:65, :], r5[64:65, :],
                                         start=True, stop=True)
                        bs = bp.tile([64, 512], F32, tag="bs")
                        nc.scalar.copy(out=bs[:], in_=p_bc[:])
                        with nc.allow_low_precision(reason="attn out f16"):
                            nc.vector.tensor_mul(out=oT_all[:, h, nsl],
                                                 in0=p_os[h][0:64, :], in1=bs[:])

                # ============ phase 3: partial out-proj + ReduceScatter ======
                s2.close()
                # Stack head pairs onto 128 partitions (DMA moves across
                # partitions; compute engines cannot).
                oT_pair = late.tile([128, 2, N], F16, tag="oTp")
                oT_r = oT_all[:].rearrange("p (q t) n -> p q t n", t=2)
                nc.sync.dma_start(out=oT_pair[0:64, :, :], in_=oT_r[:, :, 0, :])
                nc.sync.dma_start(out=oT_pair[64:128, :, :], in_=oT_r[:, :, 1, :])

                psy = s2o.enter_context(tc.tile_pool(name="psy", bufs=4, space="PSUM"))
                yp = s2o.enter_context(tc.tile_pool(name="yp", bufs=3))
                for nn in range(8):
                    for cc2 in range(2):
                        p_y = psy.tile([128, 512], F32, tag="py")
                        for q_ in range(2):
                            nc.tensor.matmul(
                                p_y[:],
                                oT_pair[:, q_, nn * 128:(nn + 1) * 128],
                                wo_sb[:, q_, cc2 * 512:(cc2 + 1) * 512],
                                start=(q_ == 0), stop=(q_ == 1))
                        y_sb = yp.tile([128, 512], F16, tag="ysb")
                        with nc.allow_low_precision(reason="partial y f16"):
                            nc.scalar.copy(out=y_sb[:], in_=p_y[:])
                        nc.sync.dma_start(
                            out=y_part[nn * 128:(nn + 1) * 128,
                                       cc2 * 512:(cc2 + 1) * 512],
                            in_=y_sb[:])

                nc.gpsimd.collective_compute(
                    "ReduceScatter", mybir.AluOpType.add,
                    replica_groups=GB,
                    ins=[y_part.opt()], outs=[y_rs.opt()])
                nc.sync.dma_start(out=y[:], in_=y_rs[:])

        _gather_all()
        for _rep in range(reps):
            _body()

    nc.compile()
    return nc


def _host_prep(x, context, mask, Wq, Wkv, Wo, qn_w, kn_w):
    """Build the 8 per-core input maps (fp16 shards, one byte -> one core)."""
    x = np.asarray(x)
    context = np.asarray(context)
    mask = np.asarray(mask)
    Wq = np.asarray(Wq, dtype=np.float32)
    Wkv = np.asarray(Wkv, dtype=np.float32)
    Wo = np.asarray(Wo, dtype=np.float32)
    qn_w = np.asarray(qn_w, dtype=np.float32)
    kn_w = np.asarray(kn_w, dtype=np.float32)

    from concurrent.futures import ThreadPoolExecutor

    Wq_r = Wq.reshape(C, H, D)
    Wkv_r = Wkv.reshape(C, 2, H, D)
    comb_w = qn_w * kn_w  # [H, D]

    def _xT(b):
        return x[b].T.astype(np.float16).reshape(-1)

    def _ctxT(b):
        return context[b].T.astype(np.float16).reshape(-1)

    def _maskP(b):
        # [M, N] -> bitpack along n, little bitorder: byte i bit j = n 8i+j
        mT = np.ascontiguousarray(mask[b].T)
        return np.packbits(mT, axis=-1, bitorder="little").reshape(-1)

    def _whg(hg):
        sl = slice(4 * hg, 4 * hg + 4)
        return (Wq_r[:, sl, :].astype(np.float16).reshape(-1),
                Wkv_r[:, 0, sl, :].astype(np.float16).reshape(-1),
                Wkv_r[:, 1, sl, :].astype(np.float16).reshape(-1),
                Wo[4 * hg * D:(4 * hg + 4) * D].astype(np.float16).reshape(-1))

    with ThreadPoolExecutor(max_workers=10) as ex:
        fx = [ex.submit(_xT, b) for b in range(B)]
        fc = [ex.submit(_ctxT, b) for b in range(B)]
        fm = [ex.submit(_maskP, b) for b in range(B)]
        fw = [ex.submit(_whg, hg) for hg in range(4)]
        xT16 = [f.result() for f in fx]
        ctxT16 = [f.result() for f in fc]
        maskP8 = [f.result() for f in fm]
        wslices = [f.result() for f in fw]
    wq_hg = [w[0] for w in wslices]
    wk_hg = [w[1] for w in wslices]
    wv_hg = [w[2] for w in wslices]
    wo_hg = [w[3] for w in wslices]

    blkones = np.zeros((128, 2), np.float32)
    blkones[0:64, 0] = 1.0
    blkones[64:128, 1] = 1.0
    blkq = np.zeros((2, 128), np.float32)
    blkq[0, 0:64] = 1.0
    blkq[1, 64:128] = 1.0

    XCH = C * N // 4
    CCH = C * M // 4
    MCH = M * N // 8 // 4
    WCH = C * NHL * D // 2

    in_maps = []
    for c in range(NCORES):
        b, hg = c // 4, c % 4
        r = c % 4       # rank within batch group
        r2 = c // 4     # rank within head-group pair
        heads = [4 * hg + i for i in range(NHL)]
        # tile layout is [t(partition), hdc, col]
        blkwk = np.zeros((2, 2, 128), np.float32)
        for hdc in range(2):
            for t in range(2):
                hglob = heads[2 * hdc + t]
                blkwk[t, hdc, 64 * t:64 * t + 64] = comb_w[hglob]
        in_maps.append({
            "xTc": xT16[b][r * XCH:(r + 1) * XCH],
            "ctxTc": ctxT16[b][r * CCH:(r + 1) * CCH],
            "maskTc": maskP8[b][r * MCH:(r + 1) * MCH],
            "wqc": wq_hg[hg][r2 * WCH:(r2 + 1) * WCH],
            "wkc": wk_hg[hg][r2 * WCH:(r2 + 1) * WCH],
            "wvc": wv_hg[hg][r2 * WCH:(r2 + 1) * WCH],
            "woc": wo_hg[hg][r2 * WCH:(r2 + 1) * WCH],
            "blkones": blkones, "blkq": blkq, "blkwk": blkwk,
        })
    return in_maps


def kernel(x, context, mask, Wq, Wkv, Wo, qn_w, kn_w):
    if "nc" not in _CACHE:
        _CACHE["nc"] = _build_program()
    nc = _CACHE["nc"]
    in_maps = _host_prep(x, context, mask, Wq, Wkv, Wo, qn_w, kn_w)
    res = run_bass_kernel_spmd(nc, in_maps, list(range(NCORES)))
    out = np.empty((B, N, C), np.float32)
    for c in range(NCORES):
        b, hg = c // 4, c % 4
        out[b, hg * NLOC:(hg + 1) * NLOC, :] = res.results[c]["y"].astype(np.float32)
    return out
